# revision 9
# baseline (speedup 1.0000x reference)
"""Trainium2 Bass kernel for the DCE (dynamic contrast-enhanced MRI) forward model.

Pipeline (per frame f of 50):
    CA   = k1[f] * x_c[0] + k2[f] * x_c[1]            (complex, 320x320)
    w    = qE1 * exp(c*CA)                             (complex exp, q folded)
    sig  = A + B / (1 - w)                             (rewritten signal model)
    out  = G @ sig @ G                                 (fftshifted ortho 2D DFT)

where G = P F P is the symmetric shifted DFT matrix, so ifft2c(sig) = G sig G.
The gather over time indices is folded into per-frame scalars k1/k2 on the host.
The constant A is dropped on device and added back on the host as a single
DC pixel (G @ (A*ones) @ G = 320*A at [160,160]).

Key numerical trick vs the obvious mapping: exp(a) is evaluated as
qE1*(((1+a/2)^2+1)/2)^2 -- two ACT Square ops -- instead of AF.Exp.  |a| <=
0.054 here, so the truncation (~a^3/24) is ~1e-6 relative.  This keeps the
whole kernel inside ONE activation table (Sin/Square/Copy all live in
trig_and_small), so there are no act-table reloads and no ordering
constraints between frames: the pipeline is free-running.

Sharding: 50 frames -> 8 cores x 7 frame slots (SPMD, padded with zero coefs).

Device kernel structure per frame:
  - bq/aq = x0 + ratio*x1 (DVE), cos/sin via Sin table (ACT, scale=+-ck1),
    p = qE1*exp(ck1*aq) via two Squares (ACT), wr/win = p*cb / p*sbn (Pool),
    sq1/sq2 = squared distances (ACT, scale/bias folded), d2 = sq1+sq2 (Pool),
    inv2 = fast reciprocal (DVE), sv = ((wr-1)*inv2, win*inv2) = S - A
    written straight into the virtual-K layout (DVE).
  - two chained complex matmul passes (fp32r, full-rate at N=320) with
    "virtual-K" stacking: 640 contraction rows (320 re + 320 im) packed into
    five full K=128 tiles.  Pass1: P1 = S.T @ G ; Pass2: out = P1.T @ G.
    The only partition-crossing moves are two 80KB SBUF->SBUF DMAs per frame
    for the mixed re/im tail tile (issued from the Pool queue, which has
    near-zero sequencer cost).
  - PSUM->SBUF copies are spread across ACT/DVE/Pool to balance engine load;
    a PE warmup stream at t=0 absorbs the p-state ramp, and frame 0's
    elementwise chain is emitted in two column-chunks so the tensor engine
    starts ~7us earlier.
"""

import sys

import numpy as np

for _p in ("/opt/trn_rl_repo", "/root/.axon_site/_ro/trn_rl_repo"):
    if _p not in sys.path:
        sys.path.insert(0, _p)

import concourse.bass as bass
import concourse.mybir as mybir
from concourse import bacc
from concourse.bass_utils import run_bass_kernel_spmd
from concourse.tile import TileContext

H = W = 320
NS = 50          # frames
NCORES = 8
FPC = 7          # frame slots per core (8*7 = 56 >= 50)
P = 128
F32 = mybir.dt.float32
F32R = mybir.dt.float32r
MSIZES = ((0, 128), (128, 128), (256, 64))   # m-tiles of the 320 output rows

# ---- signal model constants (mirrors reference fp32 arithmetic) ----
_f32 = np.float32
FA = _f32(10.0 * np.pi / 180.0)
TR = _f32(0.00487)
R1 = _f32(1.0)
R1CA = _f32(4.3)
SIG0 = _f32(100.0)
E1 = np.exp(-TR * R1, dtype=np.float32)
Q = np.cos(FA, dtype=np.float32)
M0 = SIG0 * (1 - Q * E1) / (np.sin(FA) * (1 - E1))
M0T = M0 * np.sin(FA)
MST = M0T * (1 - E1) / (1 - E1 * Q)
OFFS = SIG0 - MST
C = -TR * R1CA
CONST_A = float(M0T / Q + OFFS)
CONST_B = float(-M0T * (1 - Q) / Q)
QE1 = float(Q * E1)
SQE1H = float(np.sqrt(QE1) / 2.0)     # exp-square stage-2 scale/bias

_PROGRAM = None


def _build_program():
    """Build the single SPMD NeuronCore program (same for all 8 cores)."""
    nc = bacc.Bacc("TRN2", target_bir_lowering=False, debug=False,
                   num_devices=NCORES)
    AF = mybir.ActivationFunctionType
    OP = mybir.AluOpType

    xs_d = nc.dram_tensor("xs", [4, P, 3, W], F32, kind="ExternalInput")
    gv_d = nc.dram_tensor("gv", [2, P, 5, W], F32R, kind="ExternalInput")
    coef_d = nc.dram_tensor("coef", [P, FPC, 4], F32, kind="ExternalInput")
    out_d = nc.dram_tensor("out", [FPC, 2, 3, P, W], F32, kind="ExternalOutput")

    with TileContext(nc) as tc:
        with (
            tc.tile_pool(name="const", bufs=1) as cpool,
            tc.tile_pool(name="work", bufs=1) as wpool,
            tc.tile_pool(name="sv", bufs=3) as svpool,
            tc.tile_pool(name="av", bufs=3) as avpool,
            tc.tile_pool(name="ost", bufs=2) as opool,
            tc.tile_pool(name="psum", bufs=8, space="PSUM") as pspool,
        ):
            # ---- PE warmup: absorb the p-state ramp while inputs stream ----
            zwu = cpool.tile([P, W], mybir.dt.bfloat16)
            nc.vector.memset(zwu[:], 0.0)
            NWU = 14
            for i in range(NWU):
                wps = pspool.tile([P, W], F32, name=f"wu_{i}", tag="ps")
                nc.tensor.matmul(wps[:], zwu[:, 0:P], zwu[:], start=True,
                                 stop=True)

            # DMA order: coef first, then planes in consumption order
            # (x1i/x0i feed bq, x1r/x0r feed aq), then gv for pass 1.
            coef_sb = cpool.tile([P, FPC, 4], F32)
            nc.sync.dma_start(coef_sb[:], coef_d[:])
            xs_sb = cpool.tile([P, 4, 3, W], F32)
            for pl in (3, 1, 2, 0):
                nc.sync.dma_start(xs_sb[:, pl], xs_d[pl])
            gv_sb = cpool.tile([P, 2, 5, W], F32R)
            for comp in range(2):
                nc.sync.dma_start(gv_sb[:, comp], gv_d[comp])

            bias_pi2 = cpool.tile([P, 1], F32)
            nc.vector.memset(bias_pi2[:], float(np.pi / 2))
            bias_one = cpool.tile([P, 1], F32)
            nc.vector.memset(bias_one[:], 1.0)
            bias_sqe1h = cpool.tile([P, 1], F32)
            nc.vector.memset(bias_sqe1h[:], SQE1H)

            x0r = xs_sb[:, 0]
            x0i = xs_sb[:, 1]
            x1r = xs_sb[:, 2]
            x1i = xs_sb[:, 3]

            for f in range(FPC):
                rat = coef_sb[:, f, 0:1]
                ck1 = coef_sb[:, f, 1:2]
                nck1 = coef_sb[:, f, 2:3]
                ck1h = coef_sb[:, f, 3:4]

                sv = svpool.tile([P, 5, W], F32R, name=f"sv_{f}", tag="sv")
                tail = wpool.tile([P, W], F32R, name=f"tl_{f}", tag="tl",
                                  bufs=2)

                # frame 0 is emitted in two column-chunks so the first
                # pass-1 m-tile can start while the second half computes.
                # All ops read/write natural column positions of ONE set of
                # per-frame tiles (the tile framework tracks regions).
                bq = wpool.tile([P, 3, W], F32, name=f"bq_{f}",
                                tag="bq", bufs=2)
                aq = wpool.tile([P, 3, W], F32, name=f"aq_{f}",
                                tag="aq", bufs=2)
                cb = wpool.tile([P, 3, W], F32, name=f"cb_{f}",
                                tag="cb", bufs=2)
                sbn = wpool.tile([P, 3, W], F32, name=f"sbn_{f}",
                                 tag="sbn", bufs=2)
                s1 = wpool.tile([P, 3, W], F32, name=f"s1_{f}",
                                tag="s1", bufs=2)
                p_ = wpool.tile([P, 3, W], F32, name=f"p_{f}",
                                tag="p_", bufs=2)
                wr = wpool.tile([P, 3, W], F32, name=f"wr_{f}",
                                tag="wr", bufs=3)
                win = wpool.tile([P, 3, W], F32, name=f"win_{f}",
                                 tag="win", bufs=3)
                sq1 = wpool.tile([P, 3, W], F32, name=f"sq1_{f}",
                                 tag="sq1", bufs=2)
                sq2 = wpool.tile([P, 3, W], F32, name=f"sq2_{f}",
                                 tag="sq2", bufs=2)
                d2n = wpool.tile([P, 3, W], F32, name=f"d2n_{f}",
                                 tag="d2n", bufs=2)
                inv2 = wpool.tile([P, 3, W], F32, name=f"inv2_{f}",
                                  tag="inv2", bufs=2)
                chunks = ((0, 160), (160, 320)) if f == 0 else ((0, 320),)
                for c0, c1 in chunks:
                    cs = slice(c0, c1)
                    nc.vector.scalar_tensor_tensor(
                        bq[:, :, cs], x1i[:, :, cs], rat,
                        x0i[:, :, cs], OP.mult, OP.add)
                    nc.vector.scalar_tensor_tensor(
                        aq[:, :, cs], x1r[:, :, cs], rat,
                        x0r[:, :, cs], OP.mult, OP.add)

                    nc.scalar.activation(cb[:, :, cs], bq[:, :, cs],
                                         AF.Sin, bias=bias_pi2[:],
                                         scale=ck1)
                    nc.scalar.activation(sbn[:, :, cs], bq[:, :, cs],
                                         AF.Sin, scale=nck1)

                    # p = qE1 * exp(ck1*aq), evaluated as two Squares so it
                    # stays in the Sin/Square table (|ck1*aq| <= 0.054)
                    nc.scalar.activation(s1[:, :, cs], aq[:, :, cs],
                                         AF.Square, bias=bias_one[:], scale=ck1h)
                    nc.scalar.activation(p_[:, :, cs], s1[:, :, cs],
                                         AF.Square, bias=bias_sqe1h[:], scale=SQE1H)

                    nc.gpsimd.tensor_tensor(wr[:, :, cs], p_[:, :, cs],
                                            cb[:, :, cs], OP.mult)
                    nc.gpsimd.tensor_tensor(win[:, :, cs], p_[:, :, cs],
                                            sbn[:, :, cs], OP.mult)

                    nc.scalar.activation(sq1[:, :, cs], wr[:, :, cs],
                                         AF.Square, bias=bias_one[:], scale=-1.0)
                    nc.scalar.activation(sq2[:, :, cs], win[:, :, cs],
                                         AF.Square)
                    nc.gpsimd.tensor_tensor(d2n[:, :, cs], sq1[:, :, cs],
                                            sq2[:, :, cs], OP.add)
                    nc.vector.reciprocal_approx_fast(
                        out=inv2[:, :, cs], in_=d2n[:, :, cs])

                    # sv = S - A in virtual-K layout: re planes 0,1 + tail
                    # re rows 256:320 at [0:64, 4]; im planes 2,3 + im tail
                    # staged and DMA-shifted to [64:128, 4]
                    nc.vector.scalar_tensor_tensor(
                        sv[:, 0:2, c0:c1], wr[:, 0:2, c0:c1], -1.0,
                        inv2[:, 0:2, c0:c1], OP.add, OP.mult)
                    nc.vector.scalar_tensor_tensor(
                        sv[0:64, 4, c0:c1], wr[0:64, 2, c0:c1], -1.0,
                        inv2[0:64, 2, c0:c1], OP.add, OP.mult)
                    nc.vector.tensor_tensor(
                        sv[:, 2:4, c0:c1], win[:, 0:2, c0:c1],
                        inv2[:, 0:2, c0:c1], OP.mult)
                    nc.gpsimd.tensor_tensor(
                        tail[0:64, c0:c1], win[0:64, 2, c0:c1],
                        inv2[0:64, 2, c0:c1], OP.mult)
                    nc.sync.dma_start(sv[64:128, 4, c0:c1],
                                      tail[0:64, c0:c1])

                # ---- pass 1: P1 = S.T @ G  (complex via virtual-K) ----
                # mt0/mt1: one PSUM bank per (mtile, comp).  mt2 (64 rows):
                # re lands at partitions 0:64 and im at 64:128 of ONE bank
                # (PE tile_position via out base partition), which matches the
                # virtual-K tail layout exactly -> a single full copy, no
                # partition-shift DMA.
                p1 = []
                for mt, (m0, msz) in enumerate(MSIZES[:2]):
                    pre = pspool.tile([P, W], F32, name=f"p1re_{f}_{mt}", tag="ps")
                    pim = pspool.tile([P, W], F32, name=f"p1im_{f}_{mt}", tag="ps")
                    for kt in range(5):
                        nc.tensor.matmul(pre[:msz], sv[:, kt, m0:m0 + msz],
                                         gv_sb[:, 0, kt], start=kt == 0,
                                         stop=kt == 4)
                    for kt in range(5):
                        nc.tensor.matmul(pim[:msz], sv[:, kt, m0:m0 + msz],
                                         gv_sb[:, 1, kt], start=kt == 0,
                                         stop=kt == 4)
                    p1.append((pre, pim))
                p1re2 = pspool.tile([P, W], F32, name=f"p1re2_{f}", tag="ps")
                p1im2 = pspool.tile([P, W], F32, name=f"p1im2_{f}", tag="ps")
                for kt in range(5):
                    nc.tensor.matmul(p1re2[0:64], sv[:, kt, 256:320],
                                     gv_sb[:, 0, kt], start=kt == 0,
                                     stop=kt == 4)
                for kt in range(5):
                    nc.tensor.matmul(p1im2[0:64], sv[:, kt, 256:320],
                                     gv_sb[:, 1, kt], start=kt == 0,
                                     stop=kt == 4)

                # ---- assemble A_virtual from P1 PSUM tiles ----
                av = avpool.tile([P, 5, W], F32R, name=f"av_{f}", tag="av")
                nc.scalar.copy(av[:, 0], p1[0][0][:])
                nc.vector.tensor_copy(av[:, 1], p1[1][0][:])
                nc.vector.tensor_copy(av[:, 2], p1[0][1][:])
                nc.scalar.copy(av[:, 3], p1[1][1][:])
                nc.scalar.copy(av[0:64, 4], p1re2[0:64])
                tail2 = wpool.tile([P, W], F32R, name=f"tail2_{f}", tag="tail2", bufs=2)
                nc.vector.tensor_copy(tail2[0:64], p1im2[0:64])
                nc.sync.dma_start(av[64:128, 4], tail2[0:64])

                # ---- pass 2: out = P1.T @ G -> staging -> HBM ----
                # ost regions: [re mt0, re mt1, im mt0, im mt1, mt2-merged]
                # (mt2-merged: partitions 0:64 = re rows 256:320, 64:128 = im)
                ost = opool.tile([P, 6, W], F32, name=f"ost_{f}", tag="ost")
                for mt, (m0, msz) in enumerate(MSIZES[:2]):
                    qre = pspool.tile([P, W], F32, name=f"p2re_{f}_{mt}", tag="ps")
                    qim = pspool.tile([P, W], F32, name=f"p2im_{f}_{mt}", tag="ps")
                    for kt in range(5):
                        nc.tensor.matmul(qre[:msz], av[:, kt, m0:m0 + msz],
                                         gv_sb[:, 0, kt], start=kt == 0,
                                         stop=kt == 4)
                    for kt in range(5):
                        nc.tensor.matmul(qim[:msz], av[:, kt, m0:m0 + msz],
                                         gv_sb[:, 1, kt], start=kt == 0,
                                         stop=kt == 4)
                    nc.scalar.copy(ost[:msz, mt], qre[:msz])
                    nc.vector.tensor_copy(ost[:msz, 2 + mt], qim[:msz])
                q2re2 = pspool.tile([P, W], F32, name=f"q2re2_{f}", tag="ps")
                q2im2 = pspool.tile([P, W], F32, name=f"q2im2_{f}", tag="ps")
                for kt in range(5):
                    nc.tensor.matmul(q2re2[0:64], av[:, kt, 256:320],
                                     gv_sb[:, 0, kt], start=kt == 0,
                                     stop=kt == 4)
                for kt in range(5):
                    nc.tensor.matmul(q2im2[0:64], av[:, kt, 256:320],
                                     gv_sb[:, 1, kt], start=kt == 0,
                                     stop=kt == 4)
                nc.scalar.copy(ost[0:64, 4], q2re2[0:64])
                nc.vector.tensor_copy(ost[0:64, 5], q2im2[0:64])
                # re: big [mt0|mt1] + small tail; im likewise
                nc.sync.dma_start(
                    out_d[f, 0, 0:2].rearrange("t p w -> p t w"),
                    ost[:, 0:2])
                nc.sync.dma_start(out_d[f, 0, 2, 0:64], ost[0:64, 4])
                nc.sync.dma_start(
                    out_d[f, 1, 0:2].rearrange("t p w -> p t w"),
                    ost[:, 2:4])
                nc.sync.dma_start(out_d[f, 1, 2, 0:64], ost[0:64, 5])

    nc.compile()
    return nc


def _get_program():
    global _PROGRAM
    if _PROGRAM is None:
        _PROGRAM = _build_program()
    return _PROGRAM


def _pack_rows(plane):
    """[320, W] -> [P, 3, W] with row r stored at [r % 128, r // 128]."""
    padded = np.zeros((3 * P, W), np.float32)
    padded[:H] = plane
    return np.ascontiguousarray(padded.reshape(3, P, W).transpose(1, 0, 2))


def _host_inputs(x, aifci, t_samp, sample_time):
    x = np.asarray(x, np.float32)
    aifci = np.asarray(aifci, np.float32)
    t_samp = np.asarray(t_samp, np.float32)
    st = np.asarray(sample_time, np.float32)

    k_time = np.cumsum(aifci, dtype=np.float32) * np.float32(0.1)
    idx = np.argmin(np.abs(t_samp[None, :] - st[:, None]), axis=1)
    k1 = k_time[idx]
    k2 = aifci[idx]

    xs = np.stack([
        _pack_rows(x[0, :, :, 0]),
        _pack_rows(x[0, :, :, 1]),
        _pack_rows(x[1, :, :, 0]),
        _pack_rows(x[1, :, :, 1]),
    ])

    kk = np.arange(H, dtype=np.float64)
    g = np.exp(-2j * np.pi * np.outer(kk + 160, kk + 160) / H) / np.sqrt(H)
    gr = g.real.astype(np.float32)
    gi = g.imag.astype(np.float32)
    # virtual-K row layout: [re 0:256 | im 0:256 | re 256:320 ; im 256:320]
    gvre = np.concatenate([gr[0:256], -gi[0:256], gr[256:320], -gi[256:320]])
    gvim = np.concatenate([gi[0:256], gr[0:256], gi[256:320], gr[256:320]])
    gv = np.stack([
        np.ascontiguousarray(gvre.reshape(5, P, W).transpose(1, 0, 2)),
        np.ascontiguousarray(gvim.reshape(5, P, W).transpose(1, 0, 2)),
    ])

    # per-frame scalars: [ratio, ck1, -ck1, ck1/2]
    coefs = np.zeros((NCORES, P, FPC, 4), np.float32)
    for c in range(NCORES):
        for sl in range(FPC):
            fidx = c * FPC + sl
            if fidx < NS:
                ck1 = np.float32(C) * k1[fidx]
                ck2 = np.float32(C) * k2[fidx]
                coefs[c, :, sl, 0] = ck2 / ck1 if ck1 != 0 else np.float32(0)
                coefs[c, :, sl, 1] = ck1
                coefs[c, :, sl, 2] = -ck1
                coefs[c, :, sl, 3] = ck1 / 2
    return xs, gv, coefs


def _unpack_outputs(results):
    out = np.empty((NS, H, W), np.complex64)
    dc = np.float32(CONST_A * H)   # G @ (A*ones) @ G == 320*A at [160,160]
    gb = np.float32(-CONST_B)      # |B| scale left off-device
    for c in range(NCORES):
        o = np.asarray(results[c]["out"])  # [FPC, 2, 3, P, W]
        for sl in range(FPC):
            fidx = c * FPC + sl
            if fidx >= NS:
                break
            re = gb * o[sl, 0].reshape(3 * P, W)[:H]
            im = gb * o[sl, 1].reshape(3 * P, W)[:H]
            re[160, 160] += dc
            out[fidx] = re + 1j * im
    return out


def kernel(x, aifci, t_samp, sample_time):
    xs, gv, coefs = _host_inputs(x, aifci, t_samp, sample_time)
    nc = _get_program()
    in_maps = [{"xs": xs, "gv": gv, "coef": coefs[c]} for c in range(NCORES)]
    try:
        res = run_bass_kernel_spmd(nc, in_maps, list(range(NCORES)))
    except Exception:
        # a previous process can leave a NeuronCore wedged; one retry after a
        # short pause recovers it (the runtime resets the exec unit)
        import time
        time.sleep(5)
        res = run_bass_kernel_spmd(nc, in_maps, list(range(NCORES)))
    return _unpack_outputs(res.results)
